# revision 19
# baseline (speedup 1.0000x reference)
"""MoE gating (top-8 of 64 experts) Bass/Tile kernel for 8 Trainium2 NeuronCores.

Problem: x [4, 8192, 2048] f32; gate_weight [64, 2048]; adaptive_bias [64];
expert_usage [64].
  scores = x @ W.T + bias; probs = softmax(scores); top8 (values, idx);
  renormalized top weights; usage histogram of selected experts; bias/usage EMA.

Strategy (data-parallel over tokens, per the sharding_hint):
  - Shard the 32768 tokens across 8 cores (4096 each), replicate W/bias.
  - Per core, per 512-token supergroup: DMA four x tiles [128, 2048];
    PE-transpose the 16 d-chunks of each into PSUM, DVE-copy to SBUF; then 16
    fp32 matmuls accumulate scores.T[e, t] = W_chunk @ x.T_chunk into a
    [64, 512] PSUM bank (W stationary, x.T moving — measured fp32 matmul cost
    on TRN2 is ~3.2 ns/stationary-col + ~1.2 ns/moving-row, so the big
    operand must move); PE-transpose scores.T back to [128, 64] per subtile.
  - Epilogue per 128-token tile: bias add, row max (negated), Exp on ACT
    (with row-sum accum), normalize to softmax probs, DVE max/max_index for
    top-8 values+indices, renormalize top-8 (matching the reference's +1e-8),
    and a >=p8 mask whose ones(128) x mask matmul accumulates the per-expert
    usage histogram in PSUM.
  - Softmax renormalization cancellation: top_w = p_top8 / (sum(p_top8)+eps)
    is computed from the full-softmax probs, like the reference.
  - Host: gather shards, sum the 8 histograms (the "all-reduce"), tiny EMA math.
"""

import numpy as np

import concourse.bass as bass
import concourse.mybir as mybir
import concourse.tile as tile
from concourse import bacc
from concourse.bass_utils import run_bass_kernel_spmd

DIM = 2048
N_EXPERTS = 64
TOP_K = 8
N_CORES = 8
P = 128
NCHUNK = DIM // P  # 16
BIAS_UPDATE_RATE = 0.01
EPS = 1e-8

F32 = mybir.dt.float32
F32R = mybir.dt.float32r  # fp32 bits; PE transpose runs 1.5 cyc/row vs 2.0
BF16 = mybir.dt.bfloat16
U32 = mybir.dt.uint32

# Filled by kernel() with the profiling results of the last run (if tracing).
LAST_RESULTS = None


def build_program(t_core: int, reps: int = 1):
    """Single-core Bass program, v3: scores computed transposed.

    fp32 matmul cost on TRN2 is ~3.2 ns per stationary column + ~1.2 ns per
    moving row. So the big operand (x.T) must be the MOVING one: we compute
    scores.T[e, t] = (W.T)_chunk.T @ x.T_chunk with the 64-expert W chunk as
    the stationary, accumulating [64, 512] per 512-token supergroup, then
    PE-transpose scores.T back to [128, 64] per 128-token subtile for the
    top-k epilogue. This cuts PE matmul time ~2.7x vs the [t, e] orientation.

    reps > 1 wraps the compute in an on-device For_i repeat loop (benchmark
    only; outputs stay identical).
    """
    assert t_core % (4 * P) == 0
    Q = t_core // P       # 128-token tiles (inner iterations)
    S = Q // 4            # 512-token supergroups
    E = N_EXPERTS

    nc = bacc.Bacc("TRN2", target_bir_lowering=False)

    x_d = nc.dram_tensor("x", [t_core, DIM], F32, kind="ExternalInput")
    wt_d = nc.dram_tensor("wt", [P, NCHUNK, E], F32, kind="ExternalInput")
    ident_d = nc.dram_tensor("ident", [P, P], F32, kind="ExternalInput")
    bias_d = nc.dram_tensor("bias", [1, E], F32, kind="ExternalInput")
    topw_d = nc.dram_tensor("topw", [t_core, TOP_K], F32, kind="ExternalOutput")
    topi_d = nc.dram_tensor("topi", [t_core, TOP_K], U32, kind="ExternalOutput")
    hist_d = nc.dram_tensor("hist", [1, E], F32, kind="ExternalOutput")

    # token (p, q) = p*Q + q on partition p, inner iteration q
    x_v = x_d[:].rearrange("(p q) d -> q p d", p=P)
    topw_v = topw_d[:].rearrange("(p q) k -> p (q k)", p=P)
    topi_v = topi_d[:].rearrange("(p q) k -> p (q k)", p=P)

    with tile.TileContext(nc) as tc:
        with (
            tc.tile_pool(name="consts", bufs=1) as consts,
            tc.tile_pool(name="xin", bufs=2) as xin,
            tc.tile_pool(name="xtsb", bufs=20) as xtsb,
            tc.tile_pool(name="small", bufs=4) as small,
            tc.tile_pool(name="stage", bufs=1) as stage,
            tc.tile_pool(name="ps_xt", bufs=4, space="PSUM") as ps_xt,
            tc.tile_pool(name="ps_scT", bufs=1, space="PSUM") as ps_scT,
            tc.tile_pool(name="ps_back", bufs=2, space="PSUM") as ps_back,
            tc.tile_pool(name="ps_h", bufs=1, space="PSUM") as ps_h,
        ):
            wt_sb = consts.tile([P, NCHUNK, E], F32)
            nc.scalar.dma_start(out=wt_sb, in_=wt_d[:])
            bias_sb = consts.tile([1, E], F32)
            nc.scalar.dma_start(out=bias_sb, in_=bias_d[:])
            ident_sb = consts.tile([P, P], F32)
            nc.scalar.dma_start(out=ident_sb, in_=ident_d[:])
            ones_bf = consts.tile([P, 1], BF16)
            nc.vector.memset(ones_bf, 1.0)
            ones_row = consts.tile([1, P], F32)
            nc.vector.memset(ones_row, 1.0)

            bias_ps = ps_back.tile([P, E], F32, tag="sc_ps")
            nc.tensor.matmul(bias_ps, ones_row, bias_sb, start=True, stop=True)
            bias_bc = consts.tile([P, E], F32)
            nc.vector.tensor_copy(bias_bc, bias_ps)

            hist_ps = ps_h.tile([1, E], F32)
            topw_st = stage.tile([P, Q * TOP_K], F32)
            topi_st = stage.tile([P, Q * TOP_K], U32)

            def epilogue(q, sc_ps):
                # scores = psum + adaptive bias (also moves PSUM -> SBUF)
                scores = small.tile([P, E], F32)
                nc.vector.tensor_add(scores, sc_ps, bias_bc)
                negm = small.tile([P, 1], F32)
                nc.vector.reduce_max(
                    negm, scores, axis=mybir.AxisListType.X, negate=True
                )
                probs = small.tile([P, E], F32)
                zsum = small.tile([P, 1], F32)
                nc.scalar.activation(
                    probs,
                    scores,
                    mybir.ActivationFunctionType.Exp,
                    bias=negm,
                    scale=1.0,
                    accum_out=zsum,
                )
                rz = small.tile([P, 1], F32)
                nc.vector.reciprocal(rz, zsum)
                pn = small.tile([P, E], F32)
                nc.gpsimd.tensor_scalar_mul(pn, probs, rz)

                top8 = small.tile([P, TOP_K], F32)
                nc.vector.max(out=top8, in_=pn)
                nc.vector.max_index(
                    out=topi_st[:, q * TOP_K : (q + 1) * TOP_K],
                    in_max=top8,
                    in_values=pn,
                )
                s8 = small.tile([P, 1], F32)
                nc.vector.reduce_sum(s8, top8, axis=mybir.AxisListType.X)
                s8e = small.tile([P, 1], F32)
                nc.gpsimd.tensor_scalar_add(s8e, s8, EPS)
                rs = small.tile([P, 1], F32)
                nc.vector.reciprocal(rs, s8e)
                nc.gpsimd.tensor_scalar_mul(
                    topw_st[:, q * TOP_K : (q + 1) * TOP_K], top8, rs
                )
                mask = small.tile([P, E], BF16)
                nc.gpsimd.tensor_scalar(
                    mask, pn, top8[:, TOP_K - 1 : TOP_K], None, mybir.AluOpType.is_ge
                )
                nc.tensor.matmul(
                    hist_ps, ones_bf, mask, start=(q == 0), stop=(q == Q - 1)
                )

            def supergroup(s):
                xs = []
                for r in range(4):
                    x_sb = xin.tile([P, DIM], F32, tag=f"x{r}")
                    nc.sync.dma_start(out=x_sb, in_=x_v[4 * s + r])
                    xs.append(x_sb)
                # Stage all 16 transposed chunks into SBUF first, so the 16
                # accumulating matmuls then run back-to-back on PE without a
                # DVE copy on their critical path.
                xts = []
                for c in range(NCHUNK):
                    xt_ps = ps_xt.tile([P, 4 * P], F32)
                    for r in range(4):
                        nc.tensor.transpose(
                            xt_ps[:, r * P : (r + 1) * P],
                            xs[r][:, c * P : (c + 1) * P],
                            ident_sb,
                        )
                    xt_sb = xtsb.tile([P, 4 * P], F32, tag="xt")
                    nc.vector.tensor_copy(xt_sb, xt_ps)
                    xts.append(xt_sb)
                scT_ps = ps_scT.tile([E, 4 * P], F32)
                for c in range(NCHUNK):
                    nc.tensor.matmul(
                        scT_ps,
                        wt_sb[:, c, :],
                        xts[c],
                        start=(c == 0),
                        stop=(c == NCHUNK - 1),
                    )
                scT_sb = xtsb.tile([E, 4 * P], F32, tag="scT")
                nc.vector.tensor_copy(scT_sb, scT_ps)
                for r in range(4):
                    sc_ps = ps_back.tile([P, E], F32, tag="sc_ps")
                    nc.tensor.transpose(
                        sc_ps,
                        scT_sb[:, r * P : (r + 1) * P],
                        ident_sb[:E, :E],
                    )
                    epilogue(4 * s + r, sc_ps)

            if reps > 1:
                with tc.For_i(0, reps, 1):
                    for s in range(S):
                        supergroup(s)
            else:
                for s in range(S):
                    supergroup(s)

            nc.scalar.dma_start(out=topw_v, in_=topw_st)
            nc.scalar.dma_start(out=topi_v, in_=topi_st)
            hist_sb = small.tile([1, E], F32)
            nc.vector.tensor_copy(hist_sb, hist_ps)
            nc.scalar.dma_start(out=hist_d[:], in_=hist_sb)

    nc.compile()
    return nc


def build_program_v2(t_core: int, reps: int = 1):
    """Build the single-core Bass program for a t_core-token shard.

    reps > 1 wraps the whole compute in an on-device For_i repeat loop —
    only used for benchmarking (outputs stay identical; work is repeated).
    """
    assert t_core % P == 0
    Q = t_core // P  # tokens per partition (inner iterations)

    nc = bacc.Bacc("TRN2", target_bir_lowering=False)

    # x and the transpose identity are tagged float32r (same bits as fp32):
    # the PE transpose-mode matmul streams 1.5 cyc/row for f32r vs 2.0 for f32,
    # and transpose is pure data movement so the tag is numerically irrelevant.
    x_d = nc.dram_tensor("x", [t_core, DIM], F32, kind="ExternalInput")
    wt_d = nc.dram_tensor("wt", [P, NCHUNK, N_EXPERTS], F32, kind="ExternalInput")
    ident_d = nc.dram_tensor("ident", [P, P], F32, kind="ExternalInput")
    bias_d = nc.dram_tensor("bias", [1, N_EXPERTS], F32, kind="ExternalInput")
    topw_d = nc.dram_tensor("topw", [t_core, TOP_K], F32, kind="ExternalOutput")
    topi_d = nc.dram_tensor("topi", [t_core, TOP_K], U32, kind="ExternalOutput")
    hist_d = nc.dram_tensor("hist", [1, N_EXPERTS], F32, kind="ExternalOutput")

    # Token (p, q) = p*Q + q lives on partition p, inner iteration q: each
    # partition covers a contiguous token range so the final output DMA writes
    # 1KB-contiguous per partition.
    x_v = x_d[:].rearrange("(p q) d -> q p d", p=P)
    topw_v = topw_d[:].rearrange("(p q) k -> p (q k)", p=P)
    topi_v = topi_d[:].rearrange("(p q) k -> p (q k)", p=P)

    with tile.TileContext(nc) as tc:
        with (
            tc.tile_pool(name="consts", bufs=1) as consts,
            tc.tile_pool(name="xin", bufs=3) as xin,
            tc.tile_pool(name="xtsb", bufs=6) as xtsb,
            tc.tile_pool(name="small", bufs=4) as small,
            tc.tile_pool(name="stage", bufs=1) as stage,
            tc.tile_pool(name="ps_xt", bufs=5, space="PSUM") as ps_xt,
            tc.tile_pool(name="ps_sc", bufs=2, space="PSUM") as ps_sc,
            tc.tile_pool(name="ps_h", bufs=1, space="PSUM") as ps_h,
        ):
            # One-time constants (on the scalar HWDGE queue; x loads own sync)
            wt_sb = consts.tile([P, NCHUNK, N_EXPERTS], F32)
            nc.scalar.dma_start(out=wt_sb, in_=wt_d[:])
            bias_sb = consts.tile([1, N_EXPERTS], F32)
            nc.scalar.dma_start(out=bias_sb, in_=bias_d[:])
            ident_sb = consts.tile([P, P], F32)
            nc.scalar.dma_start(out=ident_sb, in_=ident_d[:])
            ones_bf = consts.tile([P, 1], BF16)
            nc.vector.memset(ones_bf, 1.0)
            ones_row = consts.tile([1, P], F32)
            nc.vector.memset(ones_row, 1.0)

            # adaptive bias broadcast to all 128 partitions, built once:
            # ones[1,128].T @ bias[1,64] -> PSUM -> SBUF
            bias_ps = ps_sc.tile([P, N_EXPERTS], F32, tag="sc_ps")
            nc.tensor.matmul(bias_ps, ones_row, bias_sb, start=True, stop=True)
            bias_bc = consts.tile([P, N_EXPERTS], F32)
            nc.vector.tensor_copy(bias_bc, bias_ps)

            hist_ps = ps_h.tile([1, N_EXPERTS], F32)
            topw_st = stage.tile([P, Q * TOP_K], F32)
            topi_st = stage.tile([P, Q * TOP_K], U32)

            def full_pass(q):
                x_sb = xin.tile([P, DIM], F32)
                nc.sync.dma_start(out=x_sb, in_=x_v[q])

                sc_ps = ps_sc.tile([P, N_EXPERTS], F32)
                for g in range(4):  # 4 groups of 4 d-chunks; 1 PSUM bank each
                    xt_ps = ps_xt.tile([P, 4 * P], F32)
                    for j in range(4):
                        c = 4 * g + j
                        nc.tensor.transpose(
                            xt_ps[:, j * P : (j + 1) * P],
                            x_sb[:, c * P : (c + 1) * P],
                            ident_sb,
                        )
                    xt_sb = xtsb.tile([P, 4 * P], F32)
                    nc.vector.tensor_copy(xt_sb, xt_ps)
                    for j in range(4):
                        c = 4 * g + j
                        nc.tensor.matmul(
                            sc_ps,
                            xt_sb[:, j * P : (j + 1) * P],
                            wt_sb[:, c, :],
                            start=(c == 0),
                            stop=(c == NCHUNK - 1),
                        )

                # scores = psum + adaptive bias (also moves PSUM -> SBUF)
                scores = small.tile([P, N_EXPERTS], F32)
                nc.vector.tensor_add(scores, sc_ps, bias_bc)

                # negated row max for the softmax shift
                negm = small.tile([P, 1], F32)
                nc.vector.reduce_max(
                    negm, scores, axis=mybir.AxisListType.X, negate=True
                )
                probs = small.tile([P, N_EXPERTS], F32)
                zsum = small.tile([P, 1], F32)
                nc.scalar.activation(
                    probs,
                    scores,
                    mybir.ActivationFunctionType.Exp,
                    bias=negm,
                    scale=1.0,
                    accum_out=zsum,
                )
                rz = small.tile([P, 1], F32)
                nc.vector.reciprocal(rz, zsum)
                pn = small.tile([P, N_EXPERTS], F32)
                nc.gpsimd.tensor_scalar_mul(pn, probs, rz)

                top8 = small.tile([P, TOP_K], F32)
                nc.vector.max(out=top8, in_=pn)
                nc.vector.max_index(
                    out=topi_st[:, q * TOP_K : (q + 1) * TOP_K],
                    in_max=top8,
                    in_values=pn,
                )

                s8 = small.tile([P, 1], F32)
                nc.vector.reduce_sum(s8, top8, axis=mybir.AxisListType.X)
                s8e = small.tile([P, 1], F32)
                nc.gpsimd.tensor_scalar_add(s8e, s8, EPS)
                rs = small.tile([P, 1], F32)
                nc.vector.reciprocal(rs, s8e)
                nc.gpsimd.tensor_scalar_mul(
                    topw_st[:, q * TOP_K : (q + 1) * TOP_K], top8, rs
                )

                # usage histogram: mask = (p >= p8) as bf16, ones.T @ mask -> [1, 64]
                mask = small.tile([P, N_EXPERTS], BF16)
                nc.gpsimd.tensor_scalar(
                    mask, pn, top8[:, TOP_K - 1 : TOP_K], None, mybir.AluOpType.is_ge
                )
                nc.tensor.matmul(
                    hist_ps, ones_bf, mask, start=(q == 0), stop=(q == Q - 1)
                )

            if reps > 1:
                with tc.For_i(0, reps, 1):
                    for q in range(Q):
                        full_pass(q)
            else:
                for q in range(Q):
                    full_pass(q)

            nc.scalar.dma_start(out=topw_v, in_=topw_st)
            nc.scalar.dma_start(out=topi_v, in_=topi_st)
            hist_sb = small.tile([1, N_EXPERTS], F32)
            nc.vector.tensor_copy(hist_sb, hist_ps)
            nc.scalar.dma_start(out=hist_d[:], in_=hist_sb)

    nc.compile()
    return nc


_PROGRAM_CACHE: dict = {}


def _get_program(t_core: int):
    if t_core not in _PROGRAM_CACHE:
        _PROGRAM_CACHE[t_core] = build_program(t_core)
    return _PROGRAM_CACHE[t_core]


def make_in_maps(xf, gate_weight, adaptive_bias, n_cores):
    t_core = xf.shape[0] // n_cores
    wt_staged = np.ascontiguousarray(
        gate_weight.T.reshape(NCHUNK, P, N_EXPERTS).transpose(1, 0, 2)
    ).astype(np.float32, copy=False)
    bias_row = np.ascontiguousarray(adaptive_bias.reshape(1, N_EXPERTS)).astype(
        np.float32, copy=False
    )
    ident = np.eye(P, dtype=np.float32)
    in_maps = []
    for i in range(n_cores):
        in_maps.append(
            {
                "x": np.ascontiguousarray(xf[i * t_core : (i + 1) * t_core]),
                "wt": wt_staged,
                "ident": ident,
                "bias": bias_row,
            }
        )
    return in_maps


def kernel(x, gate_weight, adaptive_bias, expert_usage):
    global LAST_RESULTS
    x = np.asarray(x, dtype=np.float32)
    gate_weight = np.asarray(gate_weight, dtype=np.float32)
    adaptive_bias = np.asarray(adaptive_bias, dtype=np.float32)
    expert_usage = np.asarray(expert_usage, dtype=np.float32)

    B, S, D = x.shape
    T = B * S
    assert D == DIM
    xf = x.reshape(T, D)
    t_core = T // N_CORES

    nc = _get_program(t_core)
    in_maps = make_in_maps(xf, gate_weight, adaptive_bias, N_CORES)

    res = run_bass_kernel_spmd(nc, in_maps, core_ids=list(range(N_CORES)))
    LAST_RESULTS = res

    topw = np.concatenate([r["topw"] for r in res.results], axis=0)
    topi = np.concatenate([r["topi"] for r in res.results], axis=0)
    hist = np.sum([r["hist"][0] for r in res.results], axis=0, dtype=np.float32)

    top_w = topw.reshape(B, S, TOP_K)
    top_idx = topi.view(np.int32).reshape(B, S, TOP_K)

    # all-reduced histogram -> EMA updates (tiny [64] math, fp32 like the ref)
    usage = (hist / np.float32(T * TOP_K)).astype(np.float32)
    usage_diff = usage - np.float32(1.0 / N_EXPERTS)
    new_adaptive_bias = adaptive_bias - np.float32(BIAS_UPDATE_RATE) * usage_diff
    new_expert_usage = np.float32(0.9) * expert_usage + np.float32(0.1) * usage

    return top_w, top_idx, new_adaptive_bias, new_expert_usage


# revision 20
# speedup vs baseline: 1.2683x; 1.2683x over previous
"""MoE gating (top-8 of 64 experts) Bass/Tile kernel for 8 Trainium2 NeuronCores.

Problem: x [4, 8192, 2048] f32; gate_weight [64, 2048]; adaptive_bias [64];
expert_usage [64].
  scores = x @ W.T + bias; probs = softmax(scores); top8 (values, idx);
  renormalized top weights; usage histogram of selected experts; bias/usage EMA.

Strategy (data-parallel over tokens, per the sharding_hint):
  - Shard the 32768 tokens across 8 cores (4096 each), replicate W/bias.
  - Per core, per 512-token supergroup: DMA four x tiles [128, 2048];
    PE-transpose the 16 d-chunks of each into PSUM, DVE-copy to SBUF; then 16
    fp32 matmuls accumulate scores.T[e, t] = W_chunk @ x.T_chunk into a
    [64, 512] PSUM bank (W stationary, x.T moving — measured fp32 matmul cost
    on TRN2 is ~3.2 ns/stationary-col + ~1.2 ns/moving-row, so the big
    operand must move); PE-transpose scores.T back to [128, 64] per subtile.
  - Epilogue per 128-token tile: bias add, row max (negated), Exp on ACT
    (with row-sum accum), normalize to softmax probs, DVE max/max_index for
    top-8 values+indices, renormalize top-8 (matching the reference's +1e-8),
    and a >=p8 mask whose ones(128) x mask matmul accumulates the per-expert
    usage histogram in PSUM.
  - Softmax renormalization cancellation: top_w = p_top8 / (sum(p_top8)+eps)
    is computed from the full-softmax probs, like the reference.
  - Host: gather shards, sum the 8 histograms (the "all-reduce"), tiny EMA math.
"""

import numpy as np

import concourse.bass as bass
import concourse.mybir as mybir
import concourse.tile as tile
from concourse import bacc
from concourse.bass_utils import run_bass_kernel_spmd

DIM = 2048
N_EXPERTS = 64
TOP_K = 8
N_CORES = 8
P = 128
NCHUNK = DIM // P  # 16
BIAS_UPDATE_RATE = 0.01
EPS = 1e-8

F32 = mybir.dt.float32
F32R = mybir.dt.float32r  # fp32 bits; PE transpose runs 1.5 cyc/row vs 2.0
BF16 = mybir.dt.bfloat16
U32 = mybir.dt.uint32

# Filled by kernel() with the profiling results of the last run (if tracing).
LAST_RESULTS = None


def build_program(t_core: int, reps: int = 1):
    """Single-core Bass program, v3: scores computed transposed.

    fp32 matmul cost on TRN2 is ~3.2 ns per stationary column + ~1.2 ns per
    moving row. So the big operand (x.T) must be the MOVING one: we compute
    scores.T[e, t] = (W.T)_chunk.T @ x.T_chunk with the 64-expert W chunk as
    the stationary, accumulating [64, 512] per 512-token supergroup, then
    PE-transpose scores.T back to [128, 64] per 128-token subtile for the
    top-k epilogue. This cuts PE matmul time ~2.7x vs the [t, e] orientation.

    reps > 1 wraps the compute in an on-device For_i repeat loop (benchmark
    only; outputs stay identical).
    """
    assert t_core % (4 * P) == 0
    Q = t_core // P       # 128-token tiles (inner iterations)
    S = Q // 4            # 512-token supergroups
    E = N_EXPERTS

    nc = bacc.Bacc("TRN2", target_bir_lowering=False)

    x_d = nc.dram_tensor("x", [t_core, DIM], F32, kind="ExternalInput")
    wt_d = nc.dram_tensor("wt", [P, NCHUNK, E], F32, kind="ExternalInput")
    ident_d = nc.dram_tensor("ident", [P, P], F32, kind="ExternalInput")
    bias_d = nc.dram_tensor("bias", [1, E], F32, kind="ExternalInput")
    topw_d = nc.dram_tensor("topw", [t_core, TOP_K], F32, kind="ExternalOutput")
    topi_d = nc.dram_tensor("topi", [t_core, TOP_K], U32, kind="ExternalOutput")
    hist_d = nc.dram_tensor("hist", [1, E], F32, kind="ExternalOutput")

    # token (p, q) = p*Q + q on partition p, inner iteration q
    x_v = x_d[:].rearrange("(p q) d -> q p d", p=P)
    topw_v = topw_d[:].rearrange("(p q) k -> p (q k)", p=P)
    topi_v = topi_d[:].rearrange("(p q) k -> p (q k)", p=P)

    with tile.TileContext(nc) as tc:
        with (
            tc.tile_pool(name="consts", bufs=1) as consts,
            tc.tile_pool(name="xin", bufs=2) as xin,
            tc.tile_pool(name="xtsb", bufs=20) as xtsb,
            tc.tile_pool(name="small", bufs=4) as small,
            tc.tile_pool(name="stage", bufs=1) as stage,
            tc.tile_pool(name="ps_xt", bufs=4, space="PSUM") as ps_xt,
            tc.tile_pool(name="ps_scT", bufs=1, space="PSUM") as ps_scT,
            tc.tile_pool(name="ps_back", bufs=2, space="PSUM") as ps_back,
            tc.tile_pool(name="ps_h", bufs=1, space="PSUM") as ps_h,
        ):
            wt_sb = consts.tile([P, NCHUNK, E], F32)
            nc.scalar.dma_start(out=wt_sb, in_=wt_d[:])
            bias_sb = consts.tile([1, E], F32)
            nc.scalar.dma_start(out=bias_sb, in_=bias_d[:])
            ident_sb = consts.tile([P, P], F32)
            nc.scalar.dma_start(out=ident_sb, in_=ident_d[:])
            ones_bf = consts.tile([P, 1], BF16)
            nc.vector.memset(ones_bf, 1.0)
            ones_row = consts.tile([1, P], F32)
            nc.vector.memset(ones_row, 1.0)

            bias_ps = ps_back.tile([P, E], F32, tag="sc_ps")
            nc.tensor.matmul(bias_ps, ones_row, bias_sb, start=True, stop=True)
            bias_bc = consts.tile([P, E], F32)
            nc.vector.tensor_copy(bias_bc, bias_ps)

            hist_ps = ps_h.tile([1, E], F32)
            topw_st = stage.tile([P, Q * TOP_K], F32)
            topi_st = stage.tile([P, Q * TOP_K], U32)

            def epilogue(q, sc_ps):
                # scores = psum + adaptive bias (also moves PSUM -> SBUF)
                scores = small.tile([P, E], F32)
                nc.vector.tensor_add(scores, sc_ps, bias_bc)
                negm = small.tile([P, 1], F32)
                nc.vector.reduce_max(
                    negm, scores, axis=mybir.AxisListType.X, negate=True
                )
                probs = small.tile([P, E], F32)
                zsum = small.tile([P, 1], F32)
                nc.scalar.activation(
                    probs,
                    scores,
                    mybir.ActivationFunctionType.Exp,
                    bias=negm,
                    scale=1.0,
                    accum_out=zsum,
                )
                rz = small.tile([P, 1], F32)
                nc.vector.reciprocal(rz, zsum)
                pn = small.tile([P, E], F32)
                nc.vector.tensor_scalar_mul(pn, probs, rz)

                top8 = small.tile([P, TOP_K], F32)
                nc.vector.max(out=top8, in_=pn)
                nc.vector.max_index(
                    out=topi_st[:, q * TOP_K : (q + 1) * TOP_K],
                    in_max=top8,
                    in_values=pn,
                )
                s8 = small.tile([P, 1], F32)
                nc.vector.reduce_sum(s8, top8, axis=mybir.AxisListType.X)
                s8e = small.tile([P, 1], F32)
                nc.vector.tensor_scalar_add(s8e, s8, EPS)
                rs = small.tile([P, 1], F32)
                nc.vector.reciprocal(rs, s8e)
                nc.vector.tensor_scalar_mul(
                    topw_st[:, q * TOP_K : (q + 1) * TOP_K], top8, rs
                )
                mask = small.tile([P, E], BF16)
                nc.vector.tensor_scalar(
                    mask, pn, top8[:, TOP_K - 1 : TOP_K], None, mybir.AluOpType.is_ge
                )
                nc.tensor.matmul(
                    hist_ps, ones_bf, mask, start=(q == 0), stop=(q == Q - 1)
                )

            def supergroup(s):
                xs = []
                for r in range(4):
                    x_sb = xin.tile([P, DIM], F32, tag=f"x{r}")
                    eng = nc.sync if r % 2 == 0 else nc.scalar
                    eng.dma_start(out=x_sb, in_=x_v[4 * s + r])
                    xs.append(x_sb)
                # Stage all 16 transposed chunks into SBUF first, so the 16
                # accumulating matmuls then run back-to-back on PE without a
                # DVE copy on their critical path.
                xts = []
                for c in range(NCHUNK):
                    xt_ps = ps_xt.tile([P, 4 * P], F32)
                    for r in range(4):
                        nc.tensor.transpose(
                            xt_ps[:, r * P : (r + 1) * P],
                            xs[r][:, c * P : (c + 1) * P],
                            ident_sb,
                        )
                    xt_sb = xtsb.tile([P, 4 * P], F32, tag="xt")
                    nc.vector.tensor_copy(xt_sb, xt_ps)
                    xts.append(xt_sb)
                scT_ps = ps_scT.tile([E, 4 * P], F32)
                for c in range(NCHUNK):
                    nc.tensor.matmul(
                        scT_ps,
                        wt_sb[:, c, :],
                        xts[c],
                        start=(c == 0),
                        stop=(c == NCHUNK - 1),
                    )
                scT_sb = xtsb.tile([E, 4 * P], F32, tag="scT")
                nc.vector.tensor_copy(scT_sb, scT_ps)
                for r in range(4):
                    sc_ps = ps_back.tile([P, E], F32, tag="sc_ps")
                    nc.tensor.transpose(
                        sc_ps,
                        scT_sb[:, r * P : (r + 1) * P],
                        ident_sb[:E, :E],
                    )
                    epilogue(4 * s + r, sc_ps)

            if reps > 1:
                with tc.For_i(0, reps, 1):
                    for s in range(S):
                        supergroup(s)
            else:
                for s in range(S):
                    supergroup(s)

            nc.scalar.dma_start(out=topw_v, in_=topw_st)
            nc.scalar.dma_start(out=topi_v, in_=topi_st)
            hist_sb = small.tile([1, E], F32)
            nc.vector.tensor_copy(hist_sb, hist_ps)
            nc.scalar.dma_start(out=hist_d[:], in_=hist_sb)

    nc.compile()
    return nc


def build_program_v2(t_core: int, reps: int = 1):
    """Build the single-core Bass program for a t_core-token shard.

    reps > 1 wraps the whole compute in an on-device For_i repeat loop —
    only used for benchmarking (outputs stay identical; work is repeated).
    """
    assert t_core % P == 0
    Q = t_core // P  # tokens per partition (inner iterations)

    nc = bacc.Bacc("TRN2", target_bir_lowering=False)

    # x and the transpose identity are tagged float32r (same bits as fp32):
    # the PE transpose-mode matmul streams 1.5 cyc/row for f32r vs 2.0 for f32,
    # and transpose is pure data movement so the tag is numerically irrelevant.
    x_d = nc.dram_tensor("x", [t_core, DIM], F32, kind="ExternalInput")
    wt_d = nc.dram_tensor("wt", [P, NCHUNK, N_EXPERTS], F32, kind="ExternalInput")
    ident_d = nc.dram_tensor("ident", [P, P], F32, kind="ExternalInput")
    bias_d = nc.dram_tensor("bias", [1, N_EXPERTS], F32, kind="ExternalInput")
    topw_d = nc.dram_tensor("topw", [t_core, TOP_K], F32, kind="ExternalOutput")
    topi_d = nc.dram_tensor("topi", [t_core, TOP_K], U32, kind="ExternalOutput")
    hist_d = nc.dram_tensor("hist", [1, N_EXPERTS], F32, kind="ExternalOutput")

    # Token (p, q) = p*Q + q lives on partition p, inner iteration q: each
    # partition covers a contiguous token range so the final output DMA writes
    # 1KB-contiguous per partition.
    x_v = x_d[:].rearrange("(p q) d -> q p d", p=P)
    topw_v = topw_d[:].rearrange("(p q) k -> p (q k)", p=P)
    topi_v = topi_d[:].rearrange("(p q) k -> p (q k)", p=P)

    with tile.TileContext(nc) as tc:
        with (
            tc.tile_pool(name="consts", bufs=1) as consts,
            tc.tile_pool(name="xin", bufs=3) as xin,
            tc.tile_pool(name="xtsb", bufs=6) as xtsb,
            tc.tile_pool(name="small", bufs=4) as small,
            tc.tile_pool(name="stage", bufs=1) as stage,
            tc.tile_pool(name="ps_xt", bufs=5, space="PSUM") as ps_xt,
            tc.tile_pool(name="ps_sc", bufs=2, space="PSUM") as ps_sc,
            tc.tile_pool(name="ps_h", bufs=1, space="PSUM") as ps_h,
        ):
            # One-time constants (on the scalar HWDGE queue; x loads own sync)
            wt_sb = consts.tile([P, NCHUNK, N_EXPERTS], F32)
            nc.scalar.dma_start(out=wt_sb, in_=wt_d[:])
            bias_sb = consts.tile([1, N_EXPERTS], F32)
            nc.scalar.dma_start(out=bias_sb, in_=bias_d[:])
            ident_sb = consts.tile([P, P], F32)
            nc.scalar.dma_start(out=ident_sb, in_=ident_d[:])
            ones_bf = consts.tile([P, 1], BF16)
            nc.vector.memset(ones_bf, 1.0)
            ones_row = consts.tile([1, P], F32)
            nc.vector.memset(ones_row, 1.0)

            # adaptive bias broadcast to all 128 partitions, built once:
            # ones[1,128].T @ bias[1,64] -> PSUM -> SBUF
            bias_ps = ps_sc.tile([P, N_EXPERTS], F32, tag="sc_ps")
            nc.tensor.matmul(bias_ps, ones_row, bias_sb, start=True, stop=True)
            bias_bc = consts.tile([P, N_EXPERTS], F32)
            nc.vector.tensor_copy(bias_bc, bias_ps)

            hist_ps = ps_h.tile([1, N_EXPERTS], F32)
            topw_st = stage.tile([P, Q * TOP_K], F32)
            topi_st = stage.tile([P, Q * TOP_K], U32)

            def full_pass(q):
                x_sb = xin.tile([P, DIM], F32)
                nc.sync.dma_start(out=x_sb, in_=x_v[q])

                sc_ps = ps_sc.tile([P, N_EXPERTS], F32)
                for g in range(4):  # 4 groups of 4 d-chunks; 1 PSUM bank each
                    xt_ps = ps_xt.tile([P, 4 * P], F32)
                    for j in range(4):
                        c = 4 * g + j
                        nc.tensor.transpose(
                            xt_ps[:, j * P : (j + 1) * P],
                            x_sb[:, c * P : (c + 1) * P],
                            ident_sb,
                        )
                    xt_sb = xtsb.tile([P, 4 * P], F32)
                    nc.vector.tensor_copy(xt_sb, xt_ps)
                    for j in range(4):
                        c = 4 * g + j
                        nc.tensor.matmul(
                            sc_ps,
                            xt_sb[:, j * P : (j + 1) * P],
                            wt_sb[:, c, :],
                            start=(c == 0),
                            stop=(c == NCHUNK - 1),
                        )

                # scores = psum + adaptive bias (also moves PSUM -> SBUF)
                scores = small.tile([P, N_EXPERTS], F32)
                nc.vector.tensor_add(scores, sc_ps, bias_bc)

                # negated row max for the softmax shift
                negm = small.tile([P, 1], F32)
                nc.vector.reduce_max(
                    negm, scores, axis=mybir.AxisListType.X, negate=True
                )
                probs = small.tile([P, N_EXPERTS], F32)
                zsum = small.tile([P, 1], F32)
                nc.scalar.activation(
                    probs,
                    scores,
                    mybir.ActivationFunctionType.Exp,
                    bias=negm,
                    scale=1.0,
                    accum_out=zsum,
                )
                rz = small.tile([P, 1], F32)
                nc.vector.reciprocal(rz, zsum)
                pn = small.tile([P, N_EXPERTS], F32)
                nc.vector.tensor_scalar_mul(pn, probs, rz)

                top8 = small.tile([P, TOP_K], F32)
                nc.vector.max(out=top8, in_=pn)
                nc.vector.max_index(
                    out=topi_st[:, q * TOP_K : (q + 1) * TOP_K],
                    in_max=top8,
                    in_values=pn,
                )

                s8 = small.tile([P, 1], F32)
                nc.vector.reduce_sum(s8, top8, axis=mybir.AxisListType.X)
                s8e = small.tile([P, 1], F32)
                nc.vector.tensor_scalar_add(s8e, s8, EPS)
                rs = small.tile([P, 1], F32)
                nc.vector.reciprocal(rs, s8e)
                nc.vector.tensor_scalar_mul(
                    topw_st[:, q * TOP_K : (q + 1) * TOP_K], top8, rs
                )

                # usage histogram: mask = (p >= p8) as bf16, ones.T @ mask -> [1, 64]
                mask = small.tile([P, N_EXPERTS], BF16)
                nc.vector.tensor_scalar(
                    mask, pn, top8[:, TOP_K - 1 : TOP_K], None, mybir.AluOpType.is_ge
                )
                nc.tensor.matmul(
                    hist_ps, ones_bf, mask, start=(q == 0), stop=(q == Q - 1)
                )

            if reps > 1:
                with tc.For_i(0, reps, 1):
                    for q in range(Q):
                        full_pass(q)
            else:
                for q in range(Q):
                    full_pass(q)

            nc.scalar.dma_start(out=topw_v, in_=topw_st)
            nc.scalar.dma_start(out=topi_v, in_=topi_st)
            hist_sb = small.tile([1, N_EXPERTS], F32)
            nc.vector.tensor_copy(hist_sb, hist_ps)
            nc.scalar.dma_start(out=hist_d[:], in_=hist_sb)

    nc.compile()
    return nc


_PROGRAM_CACHE: dict = {}


def _get_program(t_core: int):
    if t_core not in _PROGRAM_CACHE:
        _PROGRAM_CACHE[t_core] = build_program(t_core)
    return _PROGRAM_CACHE[t_core]


def make_in_maps(xf, gate_weight, adaptive_bias, n_cores):
    t_core = xf.shape[0] // n_cores
    wt_staged = np.ascontiguousarray(
        gate_weight.T.reshape(NCHUNK, P, N_EXPERTS).transpose(1, 0, 2)
    ).astype(np.float32, copy=False)
    bias_row = np.ascontiguousarray(adaptive_bias.reshape(1, N_EXPERTS)).astype(
        np.float32, copy=False
    )
    ident = np.eye(P, dtype=np.float32)
    in_maps = []
    for i in range(n_cores):
        in_maps.append(
            {
                "x": np.ascontiguousarray(xf[i * t_core : (i + 1) * t_core]),
                "wt": wt_staged,
                "ident": ident,
                "bias": bias_row,
            }
        )
    return in_maps


def kernel(x, gate_weight, adaptive_bias, expert_usage):
    global LAST_RESULTS
    x = np.asarray(x, dtype=np.float32)
    gate_weight = np.asarray(gate_weight, dtype=np.float32)
    adaptive_bias = np.asarray(adaptive_bias, dtype=np.float32)
    expert_usage = np.asarray(expert_usage, dtype=np.float32)

    B, S, D = x.shape
    T = B * S
    assert D == DIM
    xf = x.reshape(T, D)
    t_core = T // N_CORES

    nc = _get_program(t_core)
    in_maps = make_in_maps(xf, gate_weight, adaptive_bias, N_CORES)

    res = run_bass_kernel_spmd(nc, in_maps, core_ids=list(range(N_CORES)))
    LAST_RESULTS = res

    topw = np.concatenate([r["topw"] for r in res.results], axis=0)
    topi = np.concatenate([r["topi"] for r in res.results], axis=0)
    hist = np.sum([r["hist"][0] for r in res.results], axis=0, dtype=np.float32)

    top_w = topw.reshape(B, S, TOP_K)
    top_idx = topi.view(np.int32).reshape(B, S, TOP_K)

    # all-reduced histogram -> EMA updates (tiny [64] math, fp32 like the ref)
    usage = (hist / np.float32(T * TOP_K)).astype(np.float32)
    usage_diff = usage - np.float32(1.0 / N_EXPERTS)
    new_adaptive_bias = adaptive_bias - np.float32(BIAS_UPDATE_RATE) * usage_diff
    new_expert_usage = np.float32(0.9) * expert_usage + np.float32(0.1) * usage

    return top_w, top_idx, new_adaptive_bias, new_expert_usage
